# revision 52
# baseline (speedup 1.0000x reference)
"""AGCN (Chebyshev graph conv + per-node clustered GEMM + bias + cluster-max)
distributed over 8 trn2 NeuronCores.

Full inputs in, full output out. Internally:
  - node dim (420) sharded across 8 cores (52/53 nodes each, padded to 53)
  - x replicated to every core (host-side staging; aggregation over m needs full x)
  - per-core Bass kernel:
      S1 = L, S2 = 2 L @ L - I           (computed on device, transposed layout)
      xgT[k][c, b, n] = sum_m S_k[n, m] x[b, m, c]   (b-pair matmuls, x stationary)
      out[b, n, o]   = max_cl( sum_{k,i} xgT[k][i,b,n] W[n,k,i,o,cl] + bias[n,o,cl] )
    with W streamed from HBM (memory-bound term), bias fused via a ones-row,
    and the cluster max fused into the PSUM eviction.
"""

import sys

for _p in ("/opt/trn_rl_repo",):
    if _p not in sys.path:
        sys.path.insert(0, _p)

import numpy as np

# ---------------- problem constants (hardcoded) ----------------
B = 64         # batch
NN = 420       # nodes
C = 64         # dim_in
O = 64         # dim_out
CL = 10        # cluster dim
KCH = 3        # chebyshev order
NCORES = 8
PN = 53        # padded per-core node count
BOUNDS = [(NN * i) // NCORES for i in range(NCORES + 1)]
OCL = O * CL           # 640
KI = KCH * C           # 192
ROWS = KI + 1          # 193: 192 weight rows + 1 bias row
BC = B * C             # 4096
MCH = [105, 105, 105, 105]  # contraction (m / j) chunk sizes covering 420
MR = 105                    # rows per chunk (uniform: 4*105 = 420, no pad)
NQ = (PN + 1) // 2     # 27 node-pair blocks in the output buffer

USE_BF16 = True        # compute/storage dtype for matmul operands
# W ships as fp8 e4m3 scaled by 2^11 (keeps values out of the subnormal
# range); x is pre-scaled by 2^-11 on host so xg*W lands unscaled, and
# the ones-row carries 2^-11 so the (2^11 * bias) row lands unscaled.
FP8_SCALE = 2048.0


def _np_dt():
    if USE_BF16:
        import ml_dtypes

        return np.dtype(ml_dtypes.bfloat16)
    return np.dtype(np.float32)


# ---------------- device graph ----------------
_NC_CACHE = None


def _build():
    global _NC_CACHE
    if _NC_CACHE is not None:
        return _NC_CACHE

    from contextlib import ExitStack

    import concourse.bacc as bacc
    import concourse.mybir as mybir
    import concourse.tile as tile
    from concourse.tile import add_dep_helper

    DT = mybir.dt.bfloat16 if USE_BF16 else mybir.dt.float32
    F32 = mybir.dt.float32

    nc = bacc.Bacc(
        "TRN2",
        target_bir_lowering=False,
        debug=False,
        enable_asserts=False,
        num_devices=NCORES,
    )

    xt_d = nc.dram_tensor("xt", [MR, 4 * BC], DT, kind="ExternalInput")
    xloc_d = nc.dram_tensor("xloc", [C, B * PN], DT, kind="ExternalInput")
    # sp packs every small L-derived tensor ([lt-as-sT-layout | 2L | IlocT])
    # into ONE contiguous per-partition run: small strided tensors each
    # cost ~6us of queue time as 848-3360B descriptors otherwise
    SPW = 4 * 106 + 4 * NN + 4 * PN  # 2316 cols
    sp_d = nc.dram_tensor("sp", [MR, SPW], DT, kind="ExternalInput")
    # W pre-staged host-side into the exact SBUF layout, group-major so
    # every DMA descriptor is a 4KB-aligned contiguous run per partition.
    # Groups 0-2 (16 nodes) stack as row-blocks of w1a/w2a; the 5-node
    # remainder group lives in w1b/w2b. W stays fp8 end-to-end: HWDGE
    # moves raw e4m3 bytes and the PE upconverts during the matmul
    # stream (mixed bf16 lhsT x fp8 rhs is legal), so no SWDGE cast.
    F8 = mybir.dt.float8e4
    w1a_d = nc.dram_tensor("w1a", [3 * 128, 16 * OCL], F8, kind="ExternalInput")
    w1b_d = nc.dram_tensor("w1b", [128, 5 * OCL], F8, kind="ExternalInput")
    w2a_d = nc.dram_tensor("w2a", [3 * 65, 16 * OCL], F8, kind="ExternalInput")
    w2b_d = nc.dram_tensor("w2b", [65, 5 * OCL], F8, kind="ExternalInput")
    # out[b, parity, q, o] -> node 2q+parity
    out_d = nc.dram_tensor("out", [B, 2, NQ, O], F32, kind="ExternalOutput")

    # contraction sub-chunks (chunk idx, base row within chunk, rows),
    # ordered by DMA arrival (chunks land in index order, halves per ring)
    SUBS = [(mc, 0, MCH[mc]) for mc in (0, 1, 2, 3)]

    with tile.TileContext(nc) as tc:
        with ExitStack() as ctx:
            const = ctx.enter_context(tc.tile_pool(name="const", bufs=1))
            psp = ctx.enter_context(tc.tile_pool(name="ps", bufs=4, space="PSUM"))

            xt = const.tile([128, 4 * BC], DT)          # x as [m, (b c)], 4 m-chunks
            sp = const.tile([128, SPW], DT)             # [sT | l2 | ilocT] combined
            sT = sp[:, 0 : 4 * 106]                     # per chunk: [S1T | S2T] cols
            l2 = sp[:, 4 * 106 : 4 * 106 + 4 * NN]      # 2L as [j, m], 4 j-chunks
            ilocT = sp[:, 4 * 106 + 4 * NN : SPW]       # I_loc^T, 4 m-chunks
            xg01 = const.tile([128, B * PN], DT)        # rows: k=0 (c) | k=1 (c)
            xg2 = const.tile([65, B * PN], DT)          # rows 0..63: k=2, row 64: ones
            # per-node-group output staging: each group's slab DMAs out as
            # soon as its last eviction lands, so the kernel tail is only
            # the 5-node remainder group's writeback
            GQ = [8, 8, 8, 3]                           # pairs per W group
            outg = [
                const.tile([128, gq * O], F32, name=f"outg{i}")
                for i, gq in enumerate(GQ)
            ]


            # ---- phase 0 DMA plan. Two HWDGE rings (sync / scalar) share
            # the HBM pipe at packet granularity. The sync ring boots
            # ~3us earlier, so the phase-1 tensors lead there, then xt
            # (phase-2 critical) is split across BOTH rings, then the fp8
            # W groups (w1 on sync, w2 on scalar) in consumption order.
            # xloc is only needed by phase 3, so it trails xt. ----
            # Both HWDGE rings carry IDENTICAL need-ordered streams, each
            # tensor split column-wise half per ring: the combined ~250GB/s
            # always serves the earliest-needed bytes, so nothing urgent
            # queues behind bulk. sp rides whole on sync (it boots ~3us
            # before scalar, absorbing the boot skew).
            GN = 16
            groups = [(g * GN, min(GN, PN - g * GN)) for g in range((PN + GN - 1) // GN)]
            wtiles = []
            for gi, (n0, gs) in enumerate(groups):
                # all node-groups resident for the whole kernel
                w1 = const.tile([128, GN * OCL], F8, name=f"w1_{gi}")
                w2 = const.tile([65, GN * OCL], F8, name=f"w2_{gi}")
                wtiles.append((w1, w2))

            def w_dma(gi):
                gs = groups[gi][1]
                w1, w2 = wtiles[gi]
                if gs == GN:
                    src1 = w1a_d[gi * 128 : (gi + 1) * 128, :]
                    src2 = w2a_d[gi * 65 : (gi + 1) * 65, :]
                else:
                    src1 = w1b_d[:, :]
                    src2 = w2b_d[:, :]
                h1, h2 = gs * OCL // 2, gs * OCL
                nc.sync.dma_start(out=w1[0:128, 0:h1], in_=src1[:, 0:h1])
                nc.scalar.dma_start(out=w1[0:128, h1:h2], in_=src1[:, h1:h2])
                nc.sync.dma_start(out=w2[0:65, 0:h1], in_=src2[:, 0:h1])
                nc.scalar.dma_start(out=w2[0:65, h1:h2], in_=src2[:, h1:h2])

            def xt_dma(c):
                H = BC // 2
                for h, eng in ((0, nc.sync), (1, nc.scalar)):
                    lo, hi = c * BC + h * H, c * BC + (h + 1) * H
                    eng.dma_start(out=xt[0:MR, lo:hi], in_=xt_d[:, lo:hi])

            # need-order: smallpack, xt, xloc, then the W groups
            nc.sync.dma_start(out=sp[0:MR, :], in_=sp_d[:, :])
            xt_dma(0)
            xt_dma(1)
            xt_dma(2)
            xt_dma(3)
            XH = (B * PN) // 2
            nc.sync.dma_start(out=xg01[0:C, 0:XH], in_=xloc_d[:, 0:XH])
            nc.scalar.dma_start(
                out=xg01[0:C, XH : B * PN], in_=xloc_d[:, XH : B * PN]
            )
            nc.gpsimd.memset(xg2[64:65, :], 1.0 / FP8_SCALE)
            w_dma(0)
            w_dma(1)
            w_dma(2)
            w_dma(3)

            # ---- phase 1: S2T[m, n] = 2 (L @ L)[n_glob, m] - I ----
            for mc in range(4):
                m = MCH[mc]
                ps = psp.tile([128, 640], F32)
                for jc in range(4):
                    jr = MCH[jc]  # contract only live j rows (l2 pad not DMA'd)
                    nc.tensor.matmul(
                        ps[0:m, 0:PN],
                        lhsT=l2[0:jr, jc * NN + mc * MR : jc * NN + mc * MR + m],
                        rhs=sT[0:jr, jc * 106 : jc * 106 + PN],
                        start=(jc == 0),
                        stop=(jc == 3),
                    )
                nc.vector.tensor_sub(
                    sT[0:m, mc * 106 + PN : mc * 106 + 106],
                    ps[0:m, 0:PN],
                    ilocT[0:m, mc * PN : (mc + 1) * PN],
                )

            # ---- phase 2: xgT for k=1,2 via b-pair matmuls (x stationary) ----
            # 4 b-pairs share one single-bank psum tile; evictions are 4 wide
            # strided CASTs per group instead of 16 narrow ones.
            xg01w = xg01[:, :].rearrange("p (pb two n) -> p pb two n", two=2, n=PN)
            xg2w = xg2[:, :].rearrange("p (pb two n) -> p pb two n", two=2, n=PN)
            for g in range(B // 8):          # 8 groups of 4 b-pairs
                p0 = g * 4
                ps = psp.tile([128, 640], F32)
                for j in range(4):
                    p = p0 + j
                    nsub = len(SUBS)
                    for si, (mc, b0, sz) in enumerate(SUBS):
                        nc.tensor.matmul(
                            ps[:, j * 106 : j * 106 + 106],
                            lhsT=xt[
                                b0 : b0 + sz, mc * BC + p * 128 : mc * BC + (p + 1) * 128
                            ],
                            rhs=sT[b0 : b0 + sz, mc * 106 : mc * 106 + 106],
                            start=(si == 0),
                            stop=(si == nsub - 1),
                        )
                psw = ps[:, 0:424].rearrange("p (j x) -> p j x", x=106)
                for h in range(2):
                    nc.vector.tensor_copy(
                        xg01w[64:128, p0 : p0 + 4, h, :],
                        psw[h * 64 : (h + 1) * 64, :, 0:PN],
                    )
                    nc.vector.tensor_copy(
                        xg2w[0:64, p0 : p0 + 4, h, :],
                        psw[h * 64 : (h + 1) * 64, :, PN:106],
                    )

            # ---- phase 3: per-node GEMM + bias + cluster max ----
            xg01v = xg01[:, :].rearrange("p (b n) -> p b n", n=PN)
            xg2v = xg2[:, :].rearrange("p (b n) -> p b n", n=PN)

            for gi, (n0, gs) in enumerate(groups):
                w1, w2 = wtiles[gi]
                local = 0
                while local < gs:
                    npair = 2 if local + 1 < gs else 1
                    ps = psp.tile([128, OCL], F32)
                    for t in range(npair):
                        node = n0 + local + t
                        li = local + t
                        tp = None if t == 0 else (0, 64)
                        pr = slice(64 * t, 64 * (t + 1))
                        l1 = xg01v[0:128, :, node : node + 1]
                        l2h = xg2v[0:65, :, node : node + 1]
                        r1 = w1[0:128, li * OCL : (li + 1) * OCL]
                        r2 = w2[0:65, li * OCL : (li + 1) * OCL]
                        nc.tensor.matmul(
                            ps[pr, 0:512], lhsT=l1, rhs=r1[:, 0:512],
                            start=True, stop=False, tile_position=tp,
                        )
                        nc.tensor.matmul(
                            ps[pr, 512:OCL], lhsT=l1, rhs=r1[:, 512:OCL],
                            start=True, stop=False, tile_position=tp,
                        )
                        nc.tensor.matmul(
                            ps[pr, 0:512], lhsT=l2h, rhs=r2[:, 0:512],
                            start=False, stop=True, tile_position=tp,
                        )
                        nc.tensor.matmul(
                            ps[pr, 512:OCL], lhsT=l2h, rhs=r2[:, 512:OCL],
                            start=False, stop=True, tile_position=tp,
                        )
                    q = (n0 + local) // 2
                    pp = 64 * npair
                    ob, qq = outg[gi], q - 8 * gi
                    nc.vector.reduce_max(
                        ob[0:pp, qq * O : (qq + 1) * O],
                        ps[0:pp, :].rearrange("p (o c) -> p o c", c=CL),
                        axis=mybir.AxisListType.X,
                    )
                    local += npair
                # group writeback: parity 0 then parity 1 (pair 26's odd
                # half is never computed -> skipped)
                q0, nq = 8 * gi, GQ[gi]
                nq1 = nq if gi < 3 else nq - 1
                eng = nc.sync if gi % 2 == 0 else nc.scalar
                eng.dma_start(
                    out=out_d[:, 0, q0 : q0 + nq, :],
                    in_=outg[gi][0:64, :].rearrange("p (q o) -> p q o", o=O),
                )
                eng = nc.scalar if gi % 2 == 0 else nc.sync
                eng.dma_start(
                    out=out_d[:, 1, q0 : q0 + nq1, :],
                    in_=outg[gi][64:128, 0 : nq1 * O].rearrange(
                        "p (q o) -> p q o", o=O
                    ),
                )



    nc.compile()
    _NC_CACHE = nc
    return nc


# ---------------- host-side sharding / staging ----------------
def prepare_in_maps(x, node_embeddings, laplacian_mx, cluster_weights_pool, bias_pool):
    x = np.ascontiguousarray(np.asarray(x, dtype=np.float32)) * (1.0 / FP8_SCALE)
    L = np.ascontiguousarray(np.asarray(laplacian_mx, dtype=np.float32))
    # clip to +-240: TRN e4m3 tops out at 240 (vs OCP e4m3fn's 448);
    # values beyond that would read back as inf/NaN on device
    W = np.clip(np.asarray(cluster_weights_pool, dtype=np.float32) * FP8_SCALE, -240, 240)
    bias = np.clip(np.asarray(bias_pool, dtype=np.float32) * FP8_SCALE, -240, 240)
    dt = _np_dt()
    import ml_dtypes

    f8 = np.dtype(ml_dtypes.float8_e4m3fn)

    def _pack(a):
        # [420, F] -> [105, 4*F] chunk-major (420 = 4*105 exactly, no pad)
        f = a.shape[1]
        return np.ascontiguousarray(
            a.reshape(4, MR, f).transpose(1, 0, 2).reshape(MR, 4 * f)
        )

    xt = _pack(x.transpose(1, 0, 2).reshape(NN, BC)).astype(dt)
    l2 = _pack(2.0 * L).astype(dt)
    Wr = W.reshape(NN, KI, OCL)
    br = bias.reshape(NN, OCL)

    in_maps = []
    for i in range(NCORES):
        o0, o1 = BOUNDS[i], BOUNDS[i + 1]
        ni = o1 - o0
        xloc = np.zeros((C, B, PN), dtype=np.float32)
        xloc[:, :, :ni] = x[:, o0:o1, :].transpose(2, 0, 1)
        lt = np.zeros((NN, 106), dtype=np.float32)  # [S1T | zeroed S2T slot]
        lt[:, :ni] = L[o0:o1, :].T
        lt = _pack(lt)
        it = np.zeros((NN, PN), dtype=np.float32)
        it[np.arange(o0, o1), np.arange(ni)] = 1.0
        it = _pack(it).astype(dt)
        # SBUF-layout W: w1[r, n*OCL+f] = Wr[n, r, f] (r<128),
        # w2 rows 0:64 = Wr rows 128:192, row 64 = bias.
        # Group-major DRAM staging: groups 0-2 (16 nodes) as stacked
        # row-blocks so every descriptor is a 4K-aligned 20480B run.
        w1 = np.zeros((128, PN, OCL), dtype=f8)
        w1[:, :ni, :] = Wr[o0:o1, 0:128, :].transpose(1, 0, 2).astype(f8)
        w2 = np.zeros((65, PN, OCL), dtype=f8)
        w2[0:64, :ni, :] = Wr[o0:o1, 128:KI, :].transpose(1, 0, 2).astype(f8)
        w2[64, :ni, :] = br[o0:o1].astype(f8)
        w1a = w1[:, 0:48, :].reshape(128, 3, 16 * OCL).transpose(1, 0, 2)
        w2a = w2[:, 0:48, :].reshape(65, 3, 16 * OCL).transpose(1, 0, 2)
        sp = np.concatenate([lt.astype(dt), l2, it], axis=1)
        in_maps.append(
            {
                "xt": xt,
                "xloc": np.ascontiguousarray(xloc.reshape(C, B * PN)).astype(dt),
                "sp": np.ascontiguousarray(sp),
                "w1a": np.ascontiguousarray(w1a.reshape(3 * 128, 16 * OCL)),
                "w1b": np.ascontiguousarray(w1[:, 48:53, :].reshape(128, 5 * OCL)),
                "w2a": np.ascontiguousarray(w2a.reshape(3 * 65, 16 * OCL)),
                "w2b": np.ascontiguousarray(w2[:, 48:53, :].reshape(65, 5 * OCL)),
            }
        )
    return in_maps


def run_device(in_maps, trace=False, **kwargs):
    from concourse.bass_utils import run_bass_kernel_spmd

    nc = _build()
    return run_bass_kernel_spmd(
        nc, in_maps, core_ids=list(range(NCORES)), trace=trace, **kwargs
    )


def assemble(results):
    out = np.zeros((B, NN, O), dtype=np.float32)
    for i in range(NCORES):
        o0, o1 = BOUNDS[i], BOUNDS[i + 1]
        ni = o1 - o0
        arr = np.asarray(results[i]["out"], dtype=np.float32)  # [B, 2, NQ, O]
        interleaved = arr.transpose(0, 2, 1, 3).reshape(B, 2 * NQ, O)
        out[:, o0:o1, :] = interleaved[:, :ni, :]
    return out


def kernel(x, node_embeddings, laplacian_mx, cluster_weights_pool, bias_pool):
    in_maps = prepare_in_maps(
        x, node_embeddings, laplacian_mx, cluster_weights_pool, bias_pool
    )
    res = run_device(in_maps, trace=False)
    return assemble(res.results)



# revision 55
# speedup vs baseline: 1.0141x; 1.0141x over previous
"""AGCN (Chebyshev graph conv + per-node clustered GEMM + bias + cluster-max)
distributed over 8 trn2 NeuronCores.

Full inputs in, full output out. Internally:
  - node dim (420) sharded across 8 cores (52/53 nodes each, padded to 53)
  - x replicated to every core (host-side staging; aggregation over m needs full x)
  - per-core Bass kernel:
      S1 = L, S2 = 2 L @ L - I           (computed on device, transposed layout)
      xgT[k][c, b, n] = sum_m S_k[n, m] x[b, m, c]   (b-pair matmuls, x stationary)
      out[b, n, o]   = max_cl( sum_{k,i} xgT[k][i,b,n] W[n,k,i,o,cl] + bias[n,o,cl] )
    with W streamed from HBM (memory-bound term), bias fused via a ones-row,
    and the cluster max fused into the PSUM eviction.
"""

import sys

for _p in ("/opt/trn_rl_repo",):
    if _p not in sys.path:
        sys.path.insert(0, _p)

import numpy as np

# ---------------- problem constants (hardcoded) ----------------
B = 64         # batch
NN = 420       # nodes
C = 64         # dim_in
O = 64         # dim_out
CL = 10        # cluster dim
KCH = 3        # chebyshev order
NCORES = 8
PN = 53        # padded per-core node count
BOUNDS = [(NN * i) // NCORES for i in range(NCORES + 1)]
OCL = O * CL           # 640
KI = KCH * C           # 192
ROWS = KI + 1          # 193: 192 weight rows + 1 bias row
BC = B * C             # 4096
MCH = [105, 105, 105, 105]  # contraction (m / j) chunk sizes covering 420
MR = 105                    # rows per chunk (uniform: 4*105 = 420, no pad)
NQ = (PN + 1) // 2     # 27 node-pair blocks in the output buffer

USE_BF16 = True        # compute/storage dtype for matmul operands
# W ships as fp8 e4m3 scaled by 2^11 (keeps values out of the subnormal
# range); x is pre-scaled by 2^-11 on host so xg*W lands unscaled, and
# the ones-row carries 2^-11 so the (2^11 * bias) row lands unscaled.
FP8_SCALE = 2048.0


def _np_dt():
    if USE_BF16:
        import ml_dtypes

        return np.dtype(ml_dtypes.bfloat16)
    return np.dtype(np.float32)


# ---------------- device graph ----------------
_NC_CACHE = None


def _build():
    global _NC_CACHE
    if _NC_CACHE is not None:
        return _NC_CACHE

    from contextlib import ExitStack

    import concourse.bacc as bacc
    import concourse.mybir as mybir
    import concourse.tile as tile
    from concourse.tile import add_dep_helper

    DT = mybir.dt.bfloat16 if USE_BF16 else mybir.dt.float32
    F32 = mybir.dt.float32

    nc = bacc.Bacc(
        "TRN2",
        target_bir_lowering=False,
        debug=False,
        enable_asserts=False,
        num_devices=NCORES,
    )

    xt_d = nc.dram_tensor("xt", [MR, 4 * BC], DT, kind="ExternalInput")
    xloc_d = nc.dram_tensor("xloc", [C, B * PN], DT, kind="ExternalInput")
    # sp packs every small L-derived tensor ([lt-as-sT-layout | 2L | IlocT])
    # into ONE contiguous per-partition run: small strided tensors each
    # cost ~6us of queue time as 848-3360B descriptors otherwise
    SPW = 4 * 106 + 4 * NN + 4 * PN  # 2316 cols
    sp_d = nc.dram_tensor("sp", [MR, SPW], DT, kind="ExternalInput")
    # W pre-staged host-side into the exact SBUF layout, group-major so
    # every DMA descriptor is a 4KB-aligned contiguous run per partition.
    # Groups 0-2 (16 nodes) stack as row-blocks of w1a/w2a; the 5-node
    # remainder group lives in w1b/w2b. W stays fp8 end-to-end: HWDGE
    # moves raw e4m3 bytes and the PE upconverts during the matmul
    # stream (mixed bf16 lhsT x fp8 rhs is legal), so no SWDGE cast.
    F8 = mybir.dt.float8e4
    w1a_d = nc.dram_tensor("w1a", [3 * 128, 16 * OCL], F8, kind="ExternalInput")
    w1b_d = nc.dram_tensor("w1b", [128, 5 * OCL], F8, kind="ExternalInput")
    w2a_d = nc.dram_tensor("w2a", [3 * 65, 16 * OCL], F8, kind="ExternalInput")
    w2b_d = nc.dram_tensor("w2b", [65, 5 * OCL], F8, kind="ExternalInput")
    # out[b, parity, q, o] -> node 2q+parity
    out_d = nc.dram_tensor("out", [B, 2, NQ, O], F32, kind="ExternalOutput")

    # contraction sub-chunks (chunk idx, base row within chunk, rows);
    # this accumulation order measured fastest on HW
    SUBS = [(mc, 0, MCH[mc]) for mc in (1, 0, 3, 2)]

    with tile.TileContext(nc) as tc:
        with ExitStack() as ctx:
            const = ctx.enter_context(tc.tile_pool(name="const", bufs=1))
            psp = ctx.enter_context(tc.tile_pool(name="ps", bufs=4, space="PSUM"))

            xt = const.tile([128, 4 * BC], DT)          # x as [m, (b c)], 4 m-chunks
            sp = const.tile([128, SPW], DT)             # [sT | l2 | ilocT] combined
            sT = sp[:, 0 : 4 * 106]                     # per chunk: [S1T | S2T] cols
            l2 = sp[:, 4 * 106 : 4 * 106 + 4 * NN]      # 2L as [j, m], 4 j-chunks
            ilocT = sp[:, 4 * 106 + 4 * NN : SPW]       # I_loc^T, 4 m-chunks
            xg01 = const.tile([128, B * PN], DT)        # rows: k=0 (c) | k=1 (c)
            xg2 = const.tile([65, B * PN], DT)          # rows 0..63: k=2, row 64: ones
            # per-node-group output staging: each group's slab DMAs out as
            # soon as its last eviction lands, so the kernel tail is only
            # the 5-node remainder group's writeback
            GQ = [8, 8, 8, 3]                           # pairs per W group
            outg = [
                const.tile([128, gq * O], F32, name=f"outg{i}")
                for i, gq in enumerate(GQ)
            ]


            # ---- phase 0 DMA plan. Two HWDGE rings (sync / scalar) share
            # the HBM pipe at packet granularity. The sync ring boots
            # ~3us earlier, so the phase-1 tensors lead there, then xt
            # (phase-2 critical) is split across BOTH rings, then the fp8
            # W groups (w1 on sync, w2 on scalar) in consumption order.
            # xloc is only needed by phase 3, so it trails xt. ----
            # Both HWDGE rings carry IDENTICAL need-ordered streams, each
            # tensor split column-wise half per ring: the combined ~250GB/s
            # always serves the earliest-needed bytes, so nothing urgent
            # queues behind bulk. sp rides whole on sync (it boots ~3us
            # before scalar, absorbing the boot skew).
            GN = 16
            groups = [(g * GN, min(GN, PN - g * GN)) for g in range((PN + GN - 1) // GN)]
            wtiles = []
            for gi, (n0, gs) in enumerate(groups):
                # all node-groups resident for the whole kernel
                w1 = const.tile([128, GN * OCL], F8, name=f"w1_{gi}")
                w2 = const.tile([65, GN * OCL], F8, name=f"w2_{gi}")
                wtiles.append((w1, w2))

            def w_dma(gi):
                gs = groups[gi][1]
                w1, w2 = wtiles[gi]
                if gs == GN:
                    src1 = w1a_d[gi * 128 : (gi + 1) * 128, :]
                    src2 = w2a_d[gi * 65 : (gi + 1) * 65, :]
                else:
                    src1 = w1b_d[:, :]
                    src2 = w2b_d[:, :]
                h1, h2 = gs * OCL // 2, gs * OCL
                nc.sync.dma_start(out=w1[0:128, 0:h1], in_=src1[:, 0:h1])
                nc.scalar.dma_start(out=w1[0:128, h1:h2], in_=src1[:, h1:h2])
                nc.sync.dma_start(out=w2[0:65, 0:h1], in_=src2[:, 0:h1])
                nc.scalar.dma_start(out=w2[0:65, h1:h2], in_=src2[:, h1:h2])

            def xt_dma(c):
                H = BC // 2
                for h, eng in ((0, nc.sync), (1, nc.scalar)):
                    lo, hi = c * BC + h * H, c * BC + (h + 1) * H
                    eng.dma_start(out=xt[0:MR, lo:hi], in_=xt_d[:, lo:hi])

            # need-order: smallpack, xt in SUBS consumption order (c1 is
            # contracted first, c2 last), W group 0 early enough that
            # phase 3 never waits on it, then xloc and W groups 1-3
            nc.sync.dma_start(out=sp[0:MR, :], in_=sp_d[:, :])
            xt_dma(1)
            xt_dma(0)
            w_dma(0)
            xt_dma(3)
            xt_dma(2)
            XH = (B * PN) // 2
            nc.sync.dma_start(out=xg01[0:C, 0:XH], in_=xloc_d[:, 0:XH])
            nc.scalar.dma_start(
                out=xg01[0:C, XH : B * PN], in_=xloc_d[:, XH : B * PN]
            )
            nc.gpsimd.memset(xg2[64:65, :], 1.0 / FP8_SCALE)
            w_dma(1)
            w_dma(2)
            w_dma(3)

            # ---- phase 1: S2T[m, n] = 2 (L @ L)[n_glob, m] - I ----
            for mc in range(4):
                m = MCH[mc]
                ps = psp.tile([128, 640], F32)
                for jc in range(4):
                    jr = MCH[jc]  # contract only live j rows (l2 pad not DMA'd)
                    nc.tensor.matmul(
                        ps[0:m, 0:PN],
                        lhsT=l2[0:jr, jc * NN + mc * MR : jc * NN + mc * MR + m],
                        rhs=sT[0:jr, jc * 106 : jc * 106 + PN],
                        start=(jc == 0),
                        stop=(jc == 3),
                    )
                nc.vector.tensor_sub(
                    sT[0:m, mc * 106 + PN : mc * 106 + 106],
                    ps[0:m, 0:PN],
                    ilocT[0:m, mc * PN : (mc + 1) * PN],
                )

            # ---- phase 2: xgT for k=1,2 via b-pair matmuls (x stationary) ----
            # 4 b-pairs share one single-bank psum tile; evictions are 4 wide
            # strided CASTs per group instead of 16 narrow ones.
            xg01w = xg01[:, :].rearrange("p (pb two n) -> p pb two n", two=2, n=PN)
            xg2w = xg2[:, :].rearrange("p (pb two n) -> p pb two n", two=2, n=PN)
            for g in range(B // 8):          # 8 groups of 4 b-pairs
                p0 = g * 4
                ps = psp.tile([128, 640], F32)
                for j in range(4):
                    p = p0 + j
                    nsub = len(SUBS)
                    for si, (mc, b0, sz) in enumerate(SUBS):
                        nc.tensor.matmul(
                            ps[:, j * 106 : j * 106 + 106],
                            lhsT=xt[
                                b0 : b0 + sz, mc * BC + p * 128 : mc * BC + (p + 1) * 128
                            ],
                            rhs=sT[b0 : b0 + sz, mc * 106 : mc * 106 + 106],
                            start=(si == 0),
                            stop=(si == nsub - 1),
                        )
                psw = ps[:, 0:424].rearrange("p (j x) -> p j x", x=106)
                for h in range(2):
                    nc.vector.tensor_copy(
                        xg01w[64:128, p0 : p0 + 4, h, :],
                        psw[h * 64 : (h + 1) * 64, :, 0:PN],
                    )
                    nc.vector.tensor_copy(
                        xg2w[0:64, p0 : p0 + 4, h, :],
                        psw[h * 64 : (h + 1) * 64, :, PN:106],
                    )

            # ---- phase 3: per-node GEMM + bias + cluster max ----
            xg01v = xg01[:, :].rearrange("p (b n) -> p b n", n=PN)
            xg2v = xg2[:, :].rearrange("p (b n) -> p b n", n=PN)

            for gi, (n0, gs) in enumerate(groups):
                w1, w2 = wtiles[gi]
                local = 0
                while local < gs:
                    npair = 2 if local + 1 < gs else 1
                    ps = psp.tile([128, OCL], F32)
                    for t in range(npair):
                        node = n0 + local + t
                        li = local + t
                        tp = None if t == 0 else (0, 64)
                        pr = slice(64 * t, 64 * (t + 1))
                        l1 = xg01v[0:128, :, node : node + 1]
                        l2h = xg2v[0:65, :, node : node + 1]
                        r1 = w1[0:128, li * OCL : (li + 1) * OCL]
                        r2 = w2[0:65, li * OCL : (li + 1) * OCL]
                        nc.tensor.matmul(
                            ps[pr, 0:512], lhsT=l1, rhs=r1[:, 0:512],
                            start=True, stop=False, tile_position=tp,
                        )
                        nc.tensor.matmul(
                            ps[pr, 512:OCL], lhsT=l1, rhs=r1[:, 512:OCL],
                            start=True, stop=False, tile_position=tp,
                        )
                        nc.tensor.matmul(
                            ps[pr, 0:512], lhsT=l2h, rhs=r2[:, 0:512],
                            start=False, stop=True, tile_position=tp,
                        )
                        nc.tensor.matmul(
                            ps[pr, 512:OCL], lhsT=l2h, rhs=r2[:, 512:OCL],
                            start=False, stop=True, tile_position=tp,
                        )
                    q = (n0 + local) // 2
                    pp = 64 * npair
                    ob, qq = outg[gi], q - 8 * gi
                    nc.vector.reduce_max(
                        ob[0:pp, qq * O : (qq + 1) * O],
                        ps[0:pp, :].rearrange("p (o c) -> p o c", c=CL),
                        axis=mybir.AxisListType.X,
                    )
                    local += npair
                # group writeback: parity 0 then parity 1 (pair 26's odd
                # half is never computed -> skipped)
                q0, nq = 8 * gi, GQ[gi]
                nq1 = nq if gi < 3 else nq - 1
                eng = nc.sync if gi % 2 == 0 else nc.scalar
                eng.dma_start(
                    out=out_d[:, 0, q0 : q0 + nq, :],
                    in_=outg[gi][0:64, :].rearrange("p (q o) -> p q o", o=O),
                )
                eng = nc.scalar if gi % 2 == 0 else nc.sync
                eng.dma_start(
                    out=out_d[:, 1, q0 : q0 + nq1, :],
                    in_=outg[gi][64:128, 0 : nq1 * O].rearrange(
                        "p (q o) -> p q o", o=O
                    ),
                )



    nc.compile()
    _NC_CACHE = nc
    return nc


# ---------------- host-side sharding / staging ----------------
def prepare_in_maps(x, node_embeddings, laplacian_mx, cluster_weights_pool, bias_pool):
    x = np.ascontiguousarray(np.asarray(x, dtype=np.float32)) * (1.0 / FP8_SCALE)
    L = np.ascontiguousarray(np.asarray(laplacian_mx, dtype=np.float32))
    # clip to +-240: TRN e4m3 tops out at 240 (vs OCP e4m3fn's 448);
    # values beyond that would read back as inf/NaN on device
    W = np.clip(np.asarray(cluster_weights_pool, dtype=np.float32) * FP8_SCALE, -240, 240)
    bias = np.clip(np.asarray(bias_pool, dtype=np.float32) * FP8_SCALE, -240, 240)
    dt = _np_dt()
    import ml_dtypes

    f8 = np.dtype(ml_dtypes.float8_e4m3fn)

    def _pack(a):
        # [420, F] -> [105, 4*F] chunk-major (420 = 4*105 exactly, no pad)
        f = a.shape[1]
        return np.ascontiguousarray(
            a.reshape(4, MR, f).transpose(1, 0, 2).reshape(MR, 4 * f)
        )

    xt = _pack(x.transpose(1, 0, 2).reshape(NN, BC)).astype(dt)
    l2 = _pack(2.0 * L).astype(dt)
    Wr = W.reshape(NN, KI, OCL)
    br = bias.reshape(NN, OCL)

    in_maps = []
    for i in range(NCORES):
        o0, o1 = BOUNDS[i], BOUNDS[i + 1]
        ni = o1 - o0
        xloc = np.zeros((C, B, PN), dtype=np.float32)
        xloc[:, :, :ni] = x[:, o0:o1, :].transpose(2, 0, 1)
        lt = np.zeros((NN, 106), dtype=np.float32)  # [S1T | zeroed S2T slot]
        lt[:, :ni] = L[o0:o1, :].T
        lt = _pack(lt)
        it = np.zeros((NN, PN), dtype=np.float32)
        it[np.arange(o0, o1), np.arange(ni)] = 1.0
        it = _pack(it).astype(dt)
        # SBUF-layout W: w1[r, n*OCL+f] = Wr[n, r, f] (r<128),
        # w2 rows 0:64 = Wr rows 128:192, row 64 = bias.
        # Group-major DRAM staging: groups 0-2 (16 nodes) as stacked
        # row-blocks so every descriptor is a 4K-aligned 20480B run.
        w1 = np.zeros((128, PN, OCL), dtype=f8)
        w1[:, :ni, :] = Wr[o0:o1, 0:128, :].transpose(1, 0, 2).astype(f8)
        w2 = np.zeros((65, PN, OCL), dtype=f8)
        w2[0:64, :ni, :] = Wr[o0:o1, 128:KI, :].transpose(1, 0, 2).astype(f8)
        w2[64, :ni, :] = br[o0:o1].astype(f8)
        w1a = w1[:, 0:48, :].reshape(128, 3, 16 * OCL).transpose(1, 0, 2)
        w2a = w2[:, 0:48, :].reshape(65, 3, 16 * OCL).transpose(1, 0, 2)
        sp = np.concatenate([lt.astype(dt), l2, it], axis=1)
        in_maps.append(
            {
                "xt": xt,
                "xloc": np.ascontiguousarray(xloc.reshape(C, B * PN)).astype(dt),
                "sp": np.ascontiguousarray(sp),
                "w1a": np.ascontiguousarray(w1a.reshape(3 * 128, 16 * OCL)),
                "w1b": np.ascontiguousarray(w1[:, 48:53, :].reshape(128, 5 * OCL)),
                "w2a": np.ascontiguousarray(w2a.reshape(3 * 65, 16 * OCL)),
                "w2b": np.ascontiguousarray(w2[:, 48:53, :].reshape(65, 5 * OCL)),
            }
        )
    return in_maps


def run_device(in_maps, trace=False, **kwargs):
    from concourse.bass_utils import run_bass_kernel_spmd

    nc = _build()
    return run_bass_kernel_spmd(
        nc, in_maps, core_ids=list(range(NCORES)), trace=trace, **kwargs
    )


def assemble(results):
    out = np.zeros((B, NN, O), dtype=np.float32)
    for i in range(NCORES):
        o0, o1 = BOUNDS[i], BOUNDS[i + 1]
        ni = o1 - o0
        arr = np.asarray(results[i]["out"], dtype=np.float32)  # [B, 2, NQ, O]
        interleaved = arr.transpose(0, 2, 1, 3).reshape(B, 2 * NQ, O)
        out[:, o0:o1, :] = interleaved[:, :ni, :]
    return out


def kernel(x, node_embeddings, laplacian_mx, cluster_weights_pool, bias_pool):
    in_maps = prepare_in_maps(
        x, node_embeddings, laplacian_mx, cluster_weights_pool, bias_pool
    )
    res = run_device(in_maps, trace=False)
    return assemble(res.results)

